# revision 1
# baseline (speedup 1.0000x reference)
"""GQA attention prefill (B=2, S=1024, D=4096, H=32, KVH=8, HD=128) on 8 TRN2
NeuronCores.

Sharding: tensor-parallel over heads. Core c owns KV head c and query heads
4c..4c+3 (GQA groups align with cores), i.e. column-shards of wq/wk/wv and the
matching row-shard of wo. Each core computes its partial `attn_c @ wo_c`
([B*S, D]); the host sums the 8 partials.

Device layouts (per core):
  xT   [D, B*S]   bf16  host-transposed activations (contraction dim on partitions)
  wq   [D, 512]   bf16  rope-permuted (even dims then odd dims within each head)
  wk   [D, 128]   bf16  rope-permuted
  wv   [D, 128]   bf16
  wo   [512, D]   bf16
  c2   [128, S]   f32   rope cos table, duplicated across the two 64-row halves
  s2   [128, S]   f32   rope sin table, [-sin; +sin]
  mt   [128,128]  f32   causal diagonal-block additive mask * sqrt(HD)   (causal)
  mt   [S, S]     bf16  full transposed additive mask * sqrt(HD)         (general)

Q/K are produced transposed ([d, tok]) straight out of the projection matmul;
scores are computed transposed ([k, q]) so softmax denominators come from a
ones-vector matmul and P^T feeds the PV matmul with no transposes anywhere.
Rope's even/odd pairing is turned into a contiguous half-swap by permuting the
weight columns; the swap itself is two SBUF->SBUF partition-block DMAs.
Softmax skips the max-subtraction (scores are O(10); exp accumulates in fp32).
"""

import math
from contextlib import ExitStack

import numpy as np
import ml_dtypes

import concourse.bass as bass
import concourse.mybir as mybir
import concourse.tile as tile
from concourse import bacc
from concourse.bass_utils import run_bass_kernel_spmd

BF16 = mybir.dt.bfloat16
F32 = mybir.dt.float32
NPBF16 = ml_dtypes.bfloat16

B, S, D, H, KVH, HD = 2, 1024, 4096, 32, 8, 128
NCORES = 8
NH = H // NCORES          # 4 query heads per core
DC = D // 128             # 32 contraction chunks
TB = 256                  # token chunk for the QKV projection
SQ = 1.0 / math.sqrt(HD)


def _chunks(q0, qend, step=512):
    qs = q0
    while qs < qend:
        nq = min(step, qend - qs)
        yield qs, nq
        qs += nq


def build_program(causal, s=S, d=D, tb=TB):
    """Build the per-core SPMD program. s/d/tb are overridable for sim tests."""
    dc = d // 128
    nkb = s // 128            # number of 128-wide key/query blocks per batch
    ntc = s // tb             # token chunks per batch
    qcols = NH * HD

    # pT packing offsets: causal keeps only k-block ki's valid q range [128ki, s)
    if causal:
        q0s = [ki * 128 for ki in range(nkb)]
    else:
        q0s = [0] * nkb
    offs, acc = [], 0
    for ki in range(nkb):
        offs.append(acc)
        acc += s - q0s[ki]
    pt_len = acc

    nc = bacc.Bacc(
        "TRN2",
        target_bir_lowering=False,
        debug=False,
        enable_asserts=False,
        num_devices=1,
    )
    xT = nc.dram_tensor("xT", [d, B * s], BF16, kind="ExternalInput").ap()
    wq = nc.dram_tensor("wq", [d, qcols], BF16, kind="ExternalInput").ap()
    wk = nc.dram_tensor("wk", [d, HD], BF16, kind="ExternalInput").ap()
    wv = nc.dram_tensor("wv", [d, HD], BF16, kind="ExternalInput").ap()
    wo = nc.dram_tensor("wo", [qcols, d], BF16, kind="ExternalInput").ap()
    c2 = nc.dram_tensor("c2", [128, s], F32, kind="ExternalInput").ap()
    s2 = nc.dram_tensor("s2", [128, s], F32, kind="ExternalInput").ap()
    if causal:
        mt = nc.dram_tensor("mt", [128, 128], F32, kind="ExternalInput").ap()
    else:
        mt = nc.dram_tensor("mt", [s, s], BF16, kind="ExternalInput").ap()
    out = nc.dram_tensor("out", [B * s, d], F32, kind="ExternalOutput").ap()

    with tile.TileContext(nc) as tc:
        with ExitStack() as ctx:
            const = ctx.enter_context(tc.tile_pool(name="const", bufs=1))
            xpool = ctx.enter_context(tc.tile_pool(name="xpool", bufs=2))
            wopool = ctx.enter_context(tc.tile_pool(name="wopool", bufs=2))
            qkv = ctx.enter_context(tc.tile_pool(name="qkv", bufs=2))
            ptp = ctx.enter_context(tc.tile_pool(name="ptp", bufs=2))
            rp = ctx.enter_context(tc.tile_pool(name="rp", bufs=3))
            small = ctx.enter_context(tc.tile_pool(name="small", bufs=2))
            oev = ctx.enter_context(tc.tile_pool(name="oev", bufs=2))
            psm = ctx.enter_context(tc.tile_pool(name="psm", bufs=3, space="PSUM"))
            pss = ctx.enter_context(tc.tile_pool(name="pss", bufs=3, space="PSUM"))

            # resident constants / weights
            c2_sb = const.tile([128, s], F32)
            nc.sync.dma_start(c2_sb[:], c2[:])
            s2_sb = const.tile([128, s], F32)
            nc.sync.dma_start(s2_sb[:], s2[:])
            if causal:
                mt_sb = const.tile([128, 128], F32)
                nc.sync.dma_start(mt_sb[:], mt[:])
            else:
                mt_sb = const.tile([128, nkb, s], BF16)
                nc.sync.dma_start(mt_sb[:], mt.rearrange("(kb p) q -> p kb q", p=128))
            ones_sb = const.tile([128, 1], BF16)
            nc.vector.memset(ones_sb[:], 1.0)
            wq_sb = const.tile([128, dc, qcols], BF16)
            nc.sync.dma_start(wq_sb[:], wq.rearrange("(c p) m -> p c m", p=128))
            wk_sb = const.tile([128, dc, HD], BF16)
            nc.sync.dma_start(wk_sb[:], wk.rearrange("(c p) m -> p c m", p=128))
            wv_sb = const.tile([128, dc, HD], BF16)
            nc.sync.dma_start(wv_sb[:], wv.rearrange("(c p) m -> p c m", p=128))

            def rope(ps, tok0, w, out_slice):
                """ps: [128, w] psum with raw projected Q/K block (d-permuted).
                out = raw*c2 + halfswap(raw)*s2, written as bf16 to out_slice."""
                raw = rp.tile([128, w], BF16, tag="raw", name=f"raw_{tok0}")
                nc.scalar.copy(raw[:], ps[:, :w])
                sw = rp.tile([128, w], BF16, tag="sw", name=f"sw_{tok0}")
                nc.sync.dma_start(sw[0:64, :], raw[64:128, :])
                nc.sync.dma_start(sw[64:128, :], raw[0:64, :])
                t1 = rp.tile([128, w], F32, tag="t1", name=f"t1_{tok0}")
                nc.vector.tensor_mul(t1[:], sw[:], s2_sb[:, tok0 : tok0 + w])
                t2 = rp.tile([128, w], F32, tag="t2", name=f"t2_{tok0}")
                nc.vector.tensor_mul(t2[:], raw[:], c2_sb[:, tok0 : tok0 + w])
                nc.vector.tensor_add(out_slice, t2[:], t1[:])

            for b in range(B):
                # ---- persistent per-batch activations -------------------
                qT_sb = qkv.tile([128, NH, s], BF16, tag="qT", name=f"qT_{b}")
                kT_sb = qkv.tile([128, s], BF16, tag="kT", name=f"kT_{b}")
                v_sb = qkv.tile([128, nkb, HD], BF16, tag="v", name=f"v_{b}")
                attnT_sb = qkv.tile([128, NH, s], BF16, tag="attnT", name=f"attnT_{b}")

                # ---- phase 1+2: stream xT, project Q/K/V ----------------
                for t4 in range(ntc):
                    tok0 = t4 * tb
                    xc = xpool.tile([128, dc, tb], BF16, tag="xc", name=f"xc_{b}_{t4}")
                    nc.sync.dma_start(
                        xc[:],
                        xT[:, b * s + tok0 : b * s + tok0 + tb].rearrange(
                            "(c p) t -> p c t", p=128
                        ),
                    )
                    # K projection -> kT (transposed layout, rope applied)
                    k_ps = psm.tile([128, tb], F32, tag="mm", name=f"kps_{b}_{t4}")
                    for c in range(dc):
                        nc.tensor.matmul(
                            k_ps[:],
                            wk_sb[:, c, :],
                            xc[:, c, :],
                            start=(c == 0),
                            stop=(c == dc - 1),
                        )
                    rope(k_ps, tok0, tb, kT_sb[:, tok0 : tok0 + tb])
                    # V projection -> natural [tok, d] layout
                    for m2 in range(tb // 128):
                        v_ps = pss.tile([128, HD], F32, tag="sm", name=f"vps_{b}_{t4}_{m2}")
                        for c in range(dc):
                            nc.tensor.matmul(
                                v_ps[:],
                                xc[:, c, m2 * 128 : (m2 + 1) * 128],
                                wv_sb[:, c, :],
                                start=(c == 0),
                                stop=(c == dc - 1),
                            )
                        nc.scalar.copy(v_sb[:, (tok0 // 128) + m2, :], v_ps[:])
                    # Q projection -> qT (transposed layout, rope applied)
                    for h in range(NH):
                        q_ps = psm.tile([128, tb], F32, tag="mm", name=f"qps_{b}_{t4}_{h}")
                        for c in range(dc):
                            nc.tensor.matmul(
                                q_ps[:],
                                wq_sb[:, c, h * 128 : (h + 1) * 128],
                                xc[:, c, :],
                                start=(c == 0),
                                stop=(c == dc - 1),
                            )
                        rope(q_ps, tok0, tb, qT_sb[:, h, tok0 : tok0 + tb])

                # ---- phase 3: attention per query head ------------------
                for h in range(NH):
                    pT = ptp.tile([128, pt_len], BF16, tag="pt", name=f"pt_{b}_{h}")
                    sums = small.tile([1, s], F32, tag="sums", name=f"sums_{b}_{h}")
                    for ki in range(nkb):
                        q0 = q0s[ki]
                        for qs_, nq in _chunks(q0, s):
                            sc = psm.tile([128, 512], F32, tag="mm", name=f"sc_{b}_{h}_{ki}_{qs_}")
                            nc.tensor.matmul(
                                sc[:, :nq],
                                kT_sb[:, ki * 128 : (ki + 1) * 128],
                                qT_sb[:, h, qs_ : qs_ + nq],
                                start=True,
                                stop=True,
                            )
                            if causal:
                                if qs_ == q0:  # diagonal block
                                    nc.vector.tensor_add(
                                        sc[:, 0:128], sc[:, 0:128], mt_sb[:]
                                    )
                            else:
                                nc.vector.tensor_add(
                                    sc[:, :nq], sc[:, :nq], mt_sb[:, ki, qs_ : qs_ + nq]
                                )
                            po = offs[ki] + qs_ - q0
                            nc.scalar.activation(
                                pT[:, po : po + nq],
                                sc[:, :nq],
                                mybir.ActivationFunctionType.Exp,
                                scale=SQ,
                            )
                            sm = pss.tile([1, 512], F32, tag="sm", name=f"smps_{b}_{h}_{ki}_{qs_}")
                            nc.tensor.matmul(
                                sm[0:1, :nq],
                                ones_sb[:],
                                pT[:, po : po + nq],
                                start=True,
                                stop=True,
                            )
                            if ki == 0:
                                nc.vector.tensor_copy(
                                    sums[0:1, qs_ : qs_ + nq], sm[0:1, :nq]
                                )
                            else:
                                nc.vector.tensor_add(
                                    sums[0:1, qs_ : qs_ + nq],
                                    sums[0:1, qs_ : qs_ + nq],
                                    sm[0:1, :nq],
                                )
                    rsum = small.tile([1, s], F32, tag="rsum", bufs=1, name=f"rsum_{b}_{h}")
                    nc.vector.reciprocal(rsum[0:1, :], sums[0:1, :])
                    rb = small.tile([128, s], F32, tag="rb", bufs=1, name=f"rb_{b}_{h}")
                    nc.gpsimd.partition_broadcast(rb[:], rsum[0:1, :])
                    for qi in range(nkb):
                        o_ps = pss.tile([128, HD], F32, tag="sm", name=f"ops_{b}_{h}_{qi}")
                        kis = [k for k in range(nkb) if (not causal) or k <= qi]
                        for j, ki in enumerate(kis):
                            nc.tensor.matmul(
                                o_ps[:],
                                v_sb[:, ki, :],
                                pT[:, offs[ki] + qi * 128 - q0s[ki] : offs[ki] + qi * 128 - q0s[ki] + 128],
                                start=(j == 0),
                                stop=(j == len(kis) - 1),
                            )
                        nc.vector.tensor_mul(
                            attnT_sb[:, h, qi * 128 : (qi + 1) * 128],
                            o_ps[:],
                            rb[:, qi * 128 : (qi + 1) * 128],
                        )

                # ---- phase 4: output projection (partial over wo rows) --
                for nb in range(d // 512):
                    wo_nb = wopool.tile([128, NH, 512], BF16, tag="wo", name=f"wo_{b}_{nb}")
                    nc.sync.dma_start(
                        wo_nb[:],
                        wo.rearrange("(h p) n -> p h n", p=128)[
                            :, :, nb * 512 : (nb + 1) * 512
                        ],
                    )
                    for tbk in range(nkb):
                        o2 = psm.tile([128, 512], F32, tag="mm", name=f"o2_{b}_{nb}_{tbk}")
                        for h in range(NH):
                            nc.tensor.matmul(
                                o2[:],
                                attnT_sb[:, h, tbk * 128 : (tbk + 1) * 128],
                                wo_nb[:, h, :],
                                start=(h == 0),
                                stop=(h == NH - 1),
                            )
                        ot = oev.tile([128, 512], F32, tag="ot", name=f"ot_{b}_{nb}_{tbk}")
                        nc.scalar.copy(ot[:], o2[:])
                        nc.sync.dma_start(
                            out[
                                b * s + tbk * 128 : b * s + (tbk + 1) * 128,
                                nb * 512 : (nb + 1) * 512,
                            ],
                            ot[:],
                        )
    nc.compile()
    return nc


# ---------------------------------------------------------------------------
# host side
# ---------------------------------------------------------------------------

_PERM = np.concatenate([np.arange(0, HD, 2), np.arange(1, HD, 2)])
_CACHE = {}


def _get_program(causal):
    if causal not in _CACHE:
        _CACHE[causal] = build_program(causal)
    return _CACHE[causal]


def _is_causal(mask):
    iu = np.triu_indices(S, 1)
    il = np.tril_indices(S)
    return bool(np.all(mask[il] == 0.0) and np.all(mask[iu] < -1e8))


def make_in_maps(x, cos, sin, mask, wq, wk, wv, wo, causal):
    x = np.asarray(x, dtype=np.float32)
    cos = np.asarray(cos, dtype=np.float32)
    sin = np.asarray(sin, dtype=np.float32)
    mask = np.asarray(mask, dtype=np.float32)
    wq = np.asarray(wq, dtype=np.float32)
    wk = np.asarray(wk, dtype=np.float32)
    wv = np.asarray(wv, dtype=np.float32)
    wo = np.asarray(wo, dtype=np.float32)

    xT = np.ascontiguousarray(x.reshape(B * S, D).T).astype(NPBF16)
    c2 = np.ascontiguousarray(np.concatenate([cos.T, cos.T], 0)).astype(np.float32)
    s2 = np.ascontiguousarray(np.concatenate([-sin.T, sin.T], 0)).astype(np.float32)
    if causal:
        mt = np.ascontiguousarray(mask[:128, :128].T * math.sqrt(HD)).astype(np.float32)
    else:
        mt = np.ascontiguousarray(mask.T * math.sqrt(HD)).astype(NPBF16)

    in_maps = []
    for c in range(NCORES):
        wq_c = wq[:, c * NH * HD : (c + 1) * NH * HD].reshape(D, NH, HD)[:, :, _PERM]
        wq_c = np.ascontiguousarray(wq_c.reshape(D, NH * HD)).astype(NPBF16)
        wk_c = np.ascontiguousarray(wk[:, c * HD : (c + 1) * HD][:, _PERM]).astype(NPBF16)
        wv_c = np.ascontiguousarray(wv[:, c * HD : (c + 1) * HD]).astype(NPBF16)
        wo_c = np.ascontiguousarray(wo[c * NH * HD : (c + 1) * NH * HD, :]).astype(NPBF16)
        in_maps.append(
            {
                "xT": xT,
                "wq": wq_c,
                "wk": wk_c,
                "wv": wv_c,
                "wo": wo_c,
                "c2": c2,
                "s2": s2,
                "mt": mt,
            }
        )
    return in_maps


def run(in_maps, causal, **kwargs):
    nc = _get_program(causal)
    return run_bass_kernel_spmd(nc, in_maps, core_ids=list(range(NCORES)), **kwargs)


def kernel(x, start_pos, cos, sin, mask, wq, wk, wv, wo):
    mask = np.asarray(mask, dtype=np.float32)
    causal = _is_causal(mask)
    in_maps = make_in_maps(x, cos, sin, mask, wq, wk, wv, wo, causal)
    res = run(in_maps, causal)
    acc = np.zeros((B * S, D), dtype=np.float32)
    for c in range(NCORES):
        acc += np.asarray(res.results[c]["out"], dtype=np.float32)
    return acc.reshape(B, S, D)


# revision 8
# speedup vs baseline: 1.0411x; 1.0411x over previous
"""GQA attention prefill (B=2, S=1024, D=4096, H=32, KVH=8, HD=128) on 8 TRN2
NeuronCores.

Sharding: tensor-parallel over heads. Core c owns KV head c and query heads
4c..4c+3 (GQA groups align with cores), i.e. column-shards of wq/wk/wv and the
matching row-shard of wo. Each core computes its partial `attn_c @ wo_c`
([B*S, D]); the host sums the 8 partials.

Device layouts (per core):
  xT   [D, B*S]   bf16  host-transposed activations (contraction dim on partitions)
  wq   [D, 512]   bf16  rope-permuted (even dims then odd dims within each head)
  wk   [D, 128]   bf16  rope-permuted
  wv   [D, 128]   bf16
  wo   [512, D]   bf16
  c2   [128, S]   f32   rope cos table, duplicated across the two 64-row halves
  s2   [128, S]   f32   rope sin table, [-sin; +sin]
  mt   [128,128]  f32   causal diagonal-block additive mask * sqrt(HD)   (causal)
  mt   [S, S]     bf16  full transposed additive mask * sqrt(HD)         (general)

Q/K are produced transposed ([d, tok]) straight out of the projection matmul;
scores are computed transposed ([k, q]) so softmax denominators come from a
ones-vector matmul and P^T feeds the PV matmul with no transposes anywhere.
Rope's even/odd pairing is turned into a contiguous half-swap by permuting the
weight columns; the swap itself is two SBUF->SBUF partition-block DMAs.
Softmax skips the max-subtraction (scores are O(10); exp accumulates in fp32).
"""

import math
from contextlib import ExitStack

import numpy as np
import ml_dtypes

import concourse.bass as bass
import concourse.mybir as mybir
import concourse.tile as tile
from concourse import bacc
from concourse.bass_utils import run_bass_kernel_spmd

BF16 = mybir.dt.bfloat16
F32 = mybir.dt.float32
NPBF16 = ml_dtypes.bfloat16

B, S, D, H, KVH, HD = 2, 1024, 4096, 32, 8, 128
NCORES = 8
NH = H // NCORES          # 4 query heads per core
DC = D // 128             # 32 contraction chunks
TB = 256                  # token chunk for the QKV projection
SQ = 1.0 / math.sqrt(HD)


def _chunks(q0, qend, step=512):
    qs = q0
    while qs < qend:
        nq = min(step, qend - qs)
        yield qs, nq
        qs += nq


def build_program(causal, s=S, d=D, tb=TB):
    """Build the per-core SPMD program. s/d/tb are overridable for sim tests."""
    dc = d // 128
    nkb = s // 128            # number of 128-wide key/query blocks per batch
    ntc = s // tb             # token chunks per batch
    qcols = NH * HD

    # pT packing offsets: causal keeps only k-block ki's valid q range [128ki, s)
    if causal:
        q0s = [ki * 128 for ki in range(nkb)]
    else:
        q0s = [0] * nkb
    offs, acc = [], 0
    for ki in range(nkb):
        offs.append(acc)
        acc += s - q0s[ki]
    pt_len = acc

    nc = bacc.Bacc(
        "TRN2",
        target_bir_lowering=False,
        debug=False,
        enable_asserts=False,
        num_devices=1,
    )
    xT = nc.dram_tensor("xT", [d, B * s], BF16, kind="ExternalInput").ap()
    wq = nc.dram_tensor("wq", [d, qcols], BF16, kind="ExternalInput").ap()
    wk = nc.dram_tensor("wk", [d, HD], BF16, kind="ExternalInput").ap()
    wv = nc.dram_tensor("wv", [d, HD], BF16, kind="ExternalInput").ap()
    wo = nc.dram_tensor("wo", [qcols, d], BF16, kind="ExternalInput").ap()
    sw = nc.dram_tensor("sw", [128, 128], BF16, kind="ExternalInput").ap()
    c2 = nc.dram_tensor("c2", [128, s], F32, kind="ExternalInput").ap()
    s2 = nc.dram_tensor("s2", [128, s], F32, kind="ExternalInput").ap()
    if causal:
        mt = nc.dram_tensor("mt", [128, 128], F32, kind="ExternalInput").ap()
    else:
        mt = nc.dram_tensor("mt", [s, s], BF16, kind="ExternalInput").ap()
    out = nc.dram_tensor("out", [B * s, d], F32, kind="ExternalOutput").ap()

    with tile.TileContext(nc) as tc:
        with ExitStack() as ctx:
            const = ctx.enter_context(tc.tile_pool(name="const", bufs=1))
            xpool = ctx.enter_context(tc.tile_pool(name="xpool", bufs=2))
            wopool = ctx.enter_context(tc.tile_pool(name="wopool", bufs=2))
            qkv = ctx.enter_context(tc.tile_pool(name="qkv", bufs=2))
            ptp = ctx.enter_context(tc.tile_pool(name="ptp", bufs=2))
            rp = ctx.enter_context(tc.tile_pool(name="rp", bufs=3))
            small = ctx.enter_context(tc.tile_pool(name="small", bufs=2))
            oev = ctx.enter_context(tc.tile_pool(name="oev", bufs=2))
            psm = ctx.enter_context(tc.tile_pool(name="psm", bufs=3, space="PSUM"))
            pss = ctx.enter_context(tc.tile_pool(name="pss", bufs=3, space="PSUM"))

            # resident constants / weights
            c2_sb = const.tile([128, s], F32)
            nc.sync.dma_start(c2_sb[:], c2[:])
            s2_sb = const.tile([128, s], F32)
            nc.sync.dma_start(s2_sb[:], s2[:])
            if causal:
                mt_sb = const.tile([128, 128], F32)
                nc.sync.dma_start(mt_sb[:], mt[:])
            else:
                mt_sb = const.tile([128, nkb, s], BF16)
                nc.sync.dma_start(mt_sb[:], mt.rearrange("(kb p) q -> p kb q", p=128))
            ones_sb = const.tile([128, 1], BF16)
            nc.vector.memset(ones_sb[:], 1.0)
            sw_sb = const.tile([128, 128], BF16)
            nc.gpsimd.dma_start(sw_sb[:], sw[:])
            wq_sb = const.tile([128, dc, qcols], BF16)
            nc.sync.dma_start(wq_sb[:], wq.rearrange("(c p) m -> p c m", p=128))
            wk_sb = const.tile([128, dc, HD], BF16)
            nc.sync.dma_start(wk_sb[:], wk.rearrange("(c p) m -> p c m", p=128))
            wv_sb = const.tile([128, dc, HD], BF16)
            nc.sync.dma_start(wv_sb[:], wv.rearrange("(c p) m -> p c m", p=128))

            def rope(ps, tok0, w, out_slice):
                """ps: [128, w] psum with raw projected Q/K block (d-permuted).
                out = raw*c2 + halfswap(raw)*s2, written as bf16 to out_slice.
                halfswap is a stationary permutation matmul (sw_sb)."""
                raw = rp.tile([128, w], BF16, tag="raw", name=f"raw_{tok0}")
                nc.scalar.copy(raw[:], ps[:, :w])
                t2 = rp.tile([128, w], F32, tag="t2", name=f"t2_{tok0}")
                nc.vector.tensor_mul(t2[:], ps[:, :w], c2_sb[:, tok0 : tok0 + w])
                swp = psm.tile([128, w], F32, tag="mm", name=f"swp_{tok0}")
                nc.tensor.matmul(swp[:], sw_sb[:], raw[:], start=True, stop=True)
                t1 = rp.tile([128, w], F32, tag="t1", name=f"t1_{tok0}")
                nc.vector.tensor_mul(t1[:], swp[:], s2_sb[:, tok0 : tok0 + w])
                nc.gpsimd.tensor_add(out_slice, t2[:], t1[:])

            for b in range(B):
                # ---- persistent per-batch activations -------------------
                qT_sb = qkv.tile([128, NH, s], BF16, tag="qT", name=f"qT_{b}")
                kT_sb = qkv.tile([128, s], BF16, tag="kT", name=f"kT_{b}")
                v_sb = qkv.tile([128, nkb, HD], BF16, tag="v", name=f"v_{b}")
                attnT_sb = qkv.tile([128, NH, s], BF16, tag="attnT", name=f"attnT_{b}")

                # ---- phase 1+2: stream xT, project Q/K/V ----------------
                for t4 in range(ntc):
                    tok0 = t4 * tb
                    xc = xpool.tile([128, dc, tb], BF16, tag="xc", name=f"xc_{b}_{t4}")
                    nc.gpsimd.dma_start(
                        xc[:],
                        xT[:, b * s + tok0 : b * s + tok0 + tb].rearrange(
                            "(c p) t -> p c t", p=128
                        ),
                    )
                    # K projection -> kT (transposed layout, rope applied)
                    k_ps = psm.tile([128, tb], F32, tag="mm", name=f"kps_{b}_{t4}")
                    for c in range(dc):
                        nc.tensor.matmul(
                            k_ps[:],
                            wk_sb[:, c, :],
                            xc[:, c, :],
                            start=(c == 0),
                            stop=(c == dc - 1),
                        )
                    rope(k_ps, tok0, tb, kT_sb[:, tok0 : tok0 + tb])
                    # V projection -> natural [tok, d] layout
                    for m2 in range(tb // 128):
                        v_ps = pss.tile([128, HD], F32, tag="sm", name=f"vps_{b}_{t4}_{m2}")
                        for c in range(dc):
                            nc.tensor.matmul(
                                v_ps[:],
                                xc[:, c, m2 * 128 : (m2 + 1) * 128],
                                wv_sb[:, c, :],
                                start=(c == 0),
                                stop=(c == dc - 1),
                            )
                        nc.scalar.copy(v_sb[:, (tok0 // 128) + m2, :], v_ps[:])
                    # Q projection -> qT (transposed layout, rope applied)
                    for h in range(NH):
                        q_ps = psm.tile([128, tb], F32, tag="mm", name=f"qps_{b}_{t4}_{h}")
                        for c in range(dc):
                            nc.tensor.matmul(
                                q_ps[:],
                                wq_sb[:, c, h * 128 : (h + 1) * 128],
                                xc[:, c, :],
                                start=(c == 0),
                                stop=(c == dc - 1),
                            )
                        rope(q_ps, tok0, tb, qT_sb[:, h, tok0 : tok0 + tb])

                # ---- phase 3: attention per query head ------------------
                # software-pipelined: PV of head h-1 is emitted after the
                # scores/sums of head h, so the softmax-denominator chain of
                # head h-1 hides under head h's PE work.
                stage1 = {}

                def attn_scores(h):
                    pT = ptp.tile([128, pt_len], BF16, tag="pt", name=f"pt_{b}_{h}")
                    sums = pss.tile([1, s], F32, tag="sums", bufs=1, name=f"sums_{b}_{h}")
                    for ki in range(nkb):
                        q0 = q0s[ki]
                        for qs_, nq in _chunks(q0, s):
                            sc = psm.tile([128, 512], F32, tag="mm", name=f"sc_{b}_{h}_{ki}_{qs_}")
                            nc.tensor.matmul(
                                sc[:, :nq],
                                kT_sb[:, ki * 128 : (ki + 1) * 128],
                                qT_sb[:, h, qs_ : qs_ + nq],
                                start=True,
                                stop=True,
                            )
                            if causal:
                                if qs_ == q0:  # diagonal block
                                    nc.vector.tensor_add(
                                        sc[:, 0:128], sc[:, 0:128], mt_sb[:]
                                    )
                            else:
                                nc.vector.tensor_add(
                                    sc[:, :nq], sc[:, :nq], mt_sb[:, ki, qs_ : qs_ + nq]
                                )
                            po = offs[ki] + qs_ - q0
                            nc.scalar.activation(
                                pT[:, po : po + nq],
                                sc[:, :nq],
                                mybir.ActivationFunctionType.Exp,
                                scale=SQ,
                            )
                            # denominators accumulate in PSUM across ki; the
                            # causal q-ranges nest, so ki==0 (full range)
                            # starts the group for every column.
                            nc.tensor.matmul(
                                sums[0:1, qs_ : qs_ + nq],
                                ones_sb[:],
                                pT[:, po : po + nq],
                                start=(ki == 0),
                                stop=(ki == nkb - 1),
                                skip_group_check=True,
                            )
                    sums_sb = small.tile([1, s], F32, tag="sums_sb", name=f"sumsb_{b}_{h}")
                    nc.vector.tensor_copy(sums_sb[0:1, :], sums[0:1, :])
                    rb = small.tile([128, s], F32, tag="rb", name=f"rb_{b}_{h}")
                    nc.gpsimd.partition_broadcast(rb[:], sums_sb[0:1, :])
                    rbr = small.tile([128, s], F32, tag="rbr", name=f"rbr_{b}_{h}")
                    nc.vector.reciprocal_approx_fast(rbr[:], rb[:])
                    return pT, rbr

                def attn_pv(h):
                    pT, rbr = stage1.pop(h)
                    for qi in range(nkb):
                        o_ps = pss.tile([128, HD], F32, tag="sm", name=f"ops_{b}_{h}_{qi}")
                        kis = [k for k in range(nkb) if (not causal) or k <= qi]
                        for j, ki in enumerate(kis):
                            nc.tensor.matmul(
                                o_ps[:],
                                v_sb[:, ki, :],
                                pT[:, offs[ki] + qi * 128 - q0s[ki] : offs[ki] + qi * 128 - q0s[ki] + 128],
                                start=(j == 0),
                                stop=(j == len(kis) - 1),
                            )
                        nc.vector.tensor_mul(
                            attnT_sb[:, h, qi * 128 : (qi + 1) * 128],
                            o_ps[:],
                            rbr[:, qi * 128 : (qi + 1) * 128],
                        )

                for h in range(NH):
                    stage1[h] = attn_scores(h)
                    if h > 0:
                        attn_pv(h - 1)
                attn_pv(NH - 1)

                # ---- phase 4: output projection (partial over wo rows) --
                for nb in range(d // 512):
                    wo_nb = wopool.tile([128, NH, 512], BF16, tag="wo", name=f"wo_{b}_{nb}")
                    nc.gpsimd.dma_start(
                        wo_nb[:],
                        wo.rearrange("(h p) n -> p h n", p=128)[
                            :, :, nb * 512 : (nb + 1) * 512
                        ],
                    )
                    for tbk in range(nkb):
                        o2 = psm.tile([128, 512], F32, tag="mm", name=f"o2_{b}_{nb}_{tbk}")
                        for h in range(NH):
                            nc.tensor.matmul(
                                o2[:],
                                attnT_sb[:, h, tbk * 128 : (tbk + 1) * 128],
                                wo_nb[:, h, :],
                                start=(h == 0),
                                stop=(h == NH - 1),
                            )
                        ot = oev.tile([128, 512], F32, tag="ot", name=f"ot_{b}_{nb}_{tbk}")
                        if tbk % 2 == 0:
                            nc.scalar.copy(ot[:], o2[:])
                        else:
                            nc.vector.tensor_copy(ot[:], o2[:])
                        nc.sync.dma_start(
                            out[
                                b * s + tbk * 128 : b * s + (tbk + 1) * 128,
                                nb * 512 : (nb + 1) * 512,
                            ],
                            ot[:],
                        )
    nc.compile()
    return nc


# ---------------------------------------------------------------------------
# host side
# ---------------------------------------------------------------------------

_PERM = np.concatenate([np.arange(0, HD, 2), np.arange(1, HD, 2)])
_CACHE = {}


def _get_program(causal):
    if causal not in _CACHE:
        _CACHE[causal] = build_program(causal)
    return _CACHE[causal]


def _is_causal(mask):
    iu = np.triu_indices(S, 1)
    il = np.tril_indices(S)
    return bool(np.all(mask[il] == 0.0) and np.all(mask[iu] < -1e8))


def make_in_maps(x, cos, sin, mask, wq, wk, wv, wo, causal):
    x = np.asarray(x, dtype=np.float32)
    cos = np.asarray(cos, dtype=np.float32)
    sin = np.asarray(sin, dtype=np.float32)
    mask = np.asarray(mask, dtype=np.float32)
    wq = np.asarray(wq, dtype=np.float32)
    wk = np.asarray(wk, dtype=np.float32)
    wv = np.asarray(wv, dtype=np.float32)
    wo = np.asarray(wo, dtype=np.float32)

    xT = np.ascontiguousarray(x.reshape(B * S, D).T).astype(NPBF16)
    c2 = np.ascontiguousarray(np.concatenate([cos.T, cos.T], 0)).astype(np.float32)
    s2 = np.ascontiguousarray(np.concatenate([-sin.T, sin.T], 0)).astype(np.float32)
    swm = np.zeros((128, 128), dtype=np.float32)
    for j in range(128):
        swm[(j + 64) % 128, j] = 1.0
    swm = swm.astype(NPBF16)
    if causal:
        mt = np.ascontiguousarray(mask[:128, :128].T * math.sqrt(HD)).astype(np.float32)
    else:
        mt = np.ascontiguousarray(mask.T * math.sqrt(HD)).astype(NPBF16)

    in_maps = []
    for c in range(NCORES):
        wq_c = wq[:, c * NH * HD : (c + 1) * NH * HD].reshape(D, NH, HD)[:, :, _PERM]
        wq_c = np.ascontiguousarray(wq_c.reshape(D, NH * HD)).astype(NPBF16)
        wk_c = np.ascontiguousarray(wk[:, c * HD : (c + 1) * HD][:, _PERM]).astype(NPBF16)
        wv_c = np.ascontiguousarray(wv[:, c * HD : (c + 1) * HD]).astype(NPBF16)
        wo_c = np.ascontiguousarray(wo[c * NH * HD : (c + 1) * NH * HD, :]).astype(NPBF16)
        in_maps.append(
            {
                "xT": xT,
                "wq": wq_c,
                "wk": wk_c,
                "wv": wv_c,
                "wo": wo_c,
                "sw": swm,
                "c2": c2,
                "s2": s2,
                "mt": mt,
            }
        )
    return in_maps


def run(in_maps, causal, **kwargs):
    nc = _get_program(causal)
    return run_bass_kernel_spmd(nc, in_maps, core_ids=list(range(NCORES)), **kwargs)


def kernel(x, start_pos, cos, sin, mask, wq, wk, wv, wo):
    mask = np.asarray(mask, dtype=np.float32)
    causal = _is_causal(mask)
    in_maps = make_in_maps(x, cos, sin, mask, wq, wk, wv, wo, causal)
    res = run(in_maps, causal)
    acc = np.zeros((B * S, D), dtype=np.float32)
    for c in range(NCORES):
        acc += np.asarray(res.results[c]["out"], dtype=np.float32)
    return acc.reshape(B, S, D)


# revision 16
# speedup vs baseline: 1.2871x; 1.2362x over previous
"""GQA attention prefill (B=2, S=1024, D=4096, H=32, KVH=8, HD=128) on 8 TRN2
NeuronCores.

Sharding: tensor-parallel over heads. Core c owns KV head c and query heads
4c..4c+3 (GQA groups align with cores), i.e. column-shards of wq/wk/wv and the
matching row-shard of wo. Each core computes its partial `attn_c @ wo_c`
([B*S, D]); the host sums the 8 partials.

Device layouts (per core):
  xT   [D, B*S]   bf16  host-transposed activations (contraction dim on partitions)
  wq   [D, 512]   bf16  rope-permuted (even dims then odd dims within each head)
  wk   [D, 128]   bf16  rope-permuted
  wv   [D, 128]   bf16
  wo   [512, D]   bf16
  c2   [128, S]   f32   rope cos table, duplicated across the two 64-row halves
  s2   [128, S]   f32   rope sin table, [-sin; +sin]
  mt   [128,128]  f32   causal diagonal-block additive mask * sqrt(HD)   (causal)
  mt   [S, S]     bf16  full transposed additive mask * sqrt(HD)         (general)

Q/K are produced transposed ([d, tok]) straight out of the projection matmul;
scores are computed transposed ([k, q]) so softmax denominators come from a
ones-vector matmul and P^T feeds the PV matmul with no transposes anywhere.
Rope's even/odd pairing is turned into a contiguous half-swap by permuting the
weight columns; the swap itself is two SBUF->SBUF partition-block DMAs.
Softmax skips the max-subtraction (scores are O(10); exp accumulates in fp32).
"""

import math
from contextlib import ExitStack

import numpy as np
import ml_dtypes

import concourse.bass as bass
import concourse.mybir as mybir
import concourse.tile as tile
from concourse import bacc
from concourse.bass_utils import run_bass_kernel_spmd

BF16 = mybir.dt.bfloat16
F32 = mybir.dt.float32
NPBF16 = ml_dtypes.bfloat16

B, S, D, H, KVH, HD = 2, 1024, 4096, 32, 8, 128
NCORES = 8
NH = H // NCORES          # 4 query heads per core
DC = D // 128             # 32 contraction chunks
TB = 256                  # token chunk for the QKV projection
SQ = 1.0 / math.sqrt(HD)


def _chunks(q0, qend, step=512):
    qs = q0
    while qs < qend:
        nq = min(step, qend - qs)
        yield qs, nq
        qs += nq


def build_program(causal, s=S, d=D, tb=TB):
    """Build the per-core SPMD program. s/d/tb are overridable for sim tests."""
    dc = d // 128
    nkb = s // 128            # number of 128-wide key/query blocks per batch
    ntc = s // tb             # token chunks per batch
    qcols = NH * HD

    # pT packing offsets: causal keeps only k-block ki's valid q range [128ki, s)
    if causal:
        q0s = [ki * 128 for ki in range(nkb)]
    else:
        q0s = [0] * nkb
    offs, acc = [], 0
    for ki in range(nkb):
        offs.append(acc)
        acc += s - q0s[ki]
    pt_len = acc

    nc = bacc.Bacc(
        "TRN2",
        target_bir_lowering=False,
        debug=False,
        enable_asserts=False,
        num_devices=1,
    )
    xT = nc.dram_tensor("xT", [d, B * s], BF16, kind="ExternalInput").ap()
    wq = nc.dram_tensor("wq", [d, qcols], BF16, kind="ExternalInput").ap()
    wk = nc.dram_tensor("wk", [d, HD], BF16, kind="ExternalInput").ap()
    wv = nc.dram_tensor("wv", [d, HD], BF16, kind="ExternalInput").ap()
    wo = nc.dram_tensor("wo", [qcols, d], BF16, kind="ExternalInput").ap()
    sw = nc.dram_tensor("sw", [128, 128], BF16, kind="ExternalInput").ap()
    c2 = nc.dram_tensor("c2", [128, s], F32, kind="ExternalInput").ap()
    s2 = nc.dram_tensor("s2", [128, s], F32, kind="ExternalInput").ap()
    if causal:
        mt = nc.dram_tensor("mt", [128, 128], F32, kind="ExternalInput").ap()
    else:
        mt = nc.dram_tensor("mt", [s, s], BF16, kind="ExternalInput").ap()
    out = nc.dram_tensor("out", [B * s, d], F32, kind="ExternalOutput").ap()

    with tile.TileContext(nc) as tc:
        with ExitStack() as ctx:
            const = ctx.enter_context(tc.tile_pool(name="const", bufs=1))
            xpool = ctx.enter_context(tc.tile_pool(name="xpool", bufs=2))
            wopool = ctx.enter_context(tc.tile_pool(name="wopool", bufs=2))
            qkv = ctx.enter_context(tc.tile_pool(name="qkv", bufs=2))
            ptp = ctx.enter_context(tc.tile_pool(name="ptp", bufs=2))
            rp = ctx.enter_context(tc.tile_pool(name="rp", bufs=3))
            small = ctx.enter_context(tc.tile_pool(name="small", bufs=2))
            oev = ctx.enter_context(tc.tile_pool(name="oev", bufs=2))
            psm = ctx.enter_context(tc.tile_pool(name="psm", bufs=3, space="PSUM"))
            pss = ctx.enter_context(tc.tile_pool(name="pss", bufs=3, space="PSUM"))

            # resident constants / weights
            c2_sb = const.tile([128, s], F32)
            nc.sync.dma_start(c2_sb[:], c2[:])
            s2_sb = const.tile([128, s], F32)
            nc.sync.dma_start(s2_sb[:], s2[:])
            if causal:
                mt_sb = const.tile([128, 128], F32)
                nc.sync.dma_start(mt_sb[:], mt[:])
            else:
                mt_sb = const.tile([128, nkb, s], BF16)
                nc.sync.dma_start(mt_sb[:], mt.rearrange("(kb p) q -> p kb q", p=128))
            ones_sb = const.tile([128, 1], BF16)
            nc.vector.memset(ones_sb[:], 1.0)
            id_sb = const.tile([128, 128], BF16)
            nc.gpsimd.dma_start(id_sb[:], sw[:])
            wq_sb = const.tile([128, dc, qcols], BF16)
            nc.sync.dma_start(wq_sb[:], wq.rearrange("(c p) m -> p c m", p=128))
            wk_sb = const.tile([128, dc, HD], BF16)
            nc.sync.dma_start(wk_sb[:], wk.rearrange("(c p) m -> p c m", p=128))
            wv_sb = const.tile([128, dc, HD], BF16)
            nc.sync.dma_start(wv_sb[:], wv.rearrange("(c p) m -> p c m", p=128))

            def rope(ps, tok0, w, out_slice):
                """ps: [128, w] psum with raw projected Q/K block (d-permuted).
                out = raw*c2 + halfswap(raw)*s2, written as bf16 to out_slice.
                Only the ACT eviction touches PSUM; the swap is two SBUF
                partition-block DMAs and the muls run from SBUF on gpsimd/DVE."""
                raw = rp.tile([128, w], BF16, tag="raw", name=f"raw_{tok0}")
                nc.scalar.copy(raw[:], ps[:, :w])
                swt = rp.tile([128, w], BF16, tag="swt", name=f"swt_{tok0}")
                nc.sync.dma_start(swt[0:64, :], raw[64:128, :])
                nc.sync.dma_start(swt[64:128, :], raw[0:64, :])
                t1 = rp.tile([128, w], F32, tag="t1", name=f"t1_{tok0}")
                nc.vector.tensor_mul(t1[:], swt[:], s2_sb[:, tok0 : tok0 + w])
                t2 = rp.tile([128, w], F32, tag="t2", name=f"t2_{tok0}")
                nc.gpsimd.tensor_mul(t2[:], raw[:], c2_sb[:, tok0 : tok0 + w])
                nc.gpsimd.tensor_add(out_slice, t2[:], t1[:])

            for b in range(B):
                # ---- persistent per-batch activations -------------------
                qT_sb = qkv.tile([128, NH, s], BF16, tag="qT", name=f"qT_{b}")
                kT_sb = qkv.tile([128, s], BF16, tag="kT", name=f"kT_{b}")
                vT_sb = qkv.tile([128, s], BF16, tag="vT", name=f"vT_{b}")
                v_sb = qkv.tile([128, nkb, HD], BF16, tag="v", name=f"v_{b}")
                attnT_sb = qkv.tile([128, NH, s], BF16, tag="attnT", name=f"attnT_{b}")

                # ---- phase 1+2: stream xT, project Q/K/V ----------------
                for t4 in range(ntc):
                    tok0 = t4 * tb
                    xc = xpool.tile([128, dc, tb], BF16, tag="xc", name=f"xc_{b}_{t4}")
                    nc.sync.dma_start(
                        xc[:],
                        xT[:, b * s + tok0 : b * s + tok0 + tb].rearrange(
                            "(c p) t -> p c t", p=128
                        ),
                    )
                    # K projection -> kT (transposed layout, rope applied)
                    k_ps = psm.tile([128, tb], F32, tag="mm", name=f"kps_{b}_{t4}")
                    for c in range(dc):
                        nc.tensor.matmul(
                            k_ps[:],
                            wk_sb[:, c, :],
                            xc[:, c, :],
                            start=(c == 0),
                            stop=(c == dc - 1),
                        )
                    rope(k_ps, tok0, tb, kT_sb[:, tok0 : tok0 + tb])
                    # V projection, transposed like K (wide-N matmuls), then
                    # PE-transposed back to the natural [tok, d] layout
                    vt_ps = psm.tile([128, tb], F32, tag="mm", name=f"vtps_{b}_{t4}")
                    for c in range(dc):
                        nc.tensor.matmul(
                            vt_ps[:],
                            wv_sb[:, c, :],
                            xc[:, c, :],
                            start=(c == 0),
                            stop=(c == dc - 1),
                        )
                    nc.scalar.copy(vT_sb[:, tok0 : tok0 + tb], vt_ps[:])
                    for m2 in range(tb // 128):
                        kb = tok0 // 128 + m2
                        vtp = pss.tile([128, HD], BF16, tag="sm", name=f"vtp_{b}_{t4}_{m2}")
                        nc.tensor.transpose(
                            vtp[:], vT_sb[:, kb * 128 : (kb + 1) * 128], id_sb[:]
                        )
                        nc.scalar.copy(v_sb[:, kb, :], vtp[:])
                    # Q projection -> qT (transposed layout, rope applied)
                    for h in range(NH):
                        q_ps = psm.tile([128, tb], F32, tag="mm", name=f"qps_{b}_{t4}_{h}")
                        for c in range(dc):
                            nc.tensor.matmul(
                                q_ps[:],
                                wq_sb[:, c, h * 128 : (h + 1) * 128],
                                xc[:, c, :],
                                start=(c == 0),
                                stop=(c == dc - 1),
                            )
                        rope(q_ps, tok0, tb, qT_sb[:, h, tok0 : tok0 + tb])

                # ---- phase 3: attention per query head ------------------
                # software-pipelined: PV of head h-1 is emitted after the
                # scores/sums of head h, so the softmax-denominator chain of
                # head h-1 hides under head h's PE work.
                stage1 = {}

                def attn_scores(h):
                    pT = ptp.tile([128, pt_len], BF16, tag="pt", name=f"pt_{b}_{h}")
                    sums = pss.tile([1, s], F32, tag="sums", bufs=1, name=f"sums_{b}_{h}")
                    for ki in range(nkb):
                        q0 = q0s[ki]
                        for qs_, nq in _chunks(q0, s):
                            sc = psm.tile([128, 512], F32, tag="mm", name=f"sc_{b}_{h}_{ki}_{qs_}")
                            nc.tensor.matmul(
                                sc[:, :nq],
                                kT_sb[:, ki * 128 : (ki + 1) * 128],
                                qT_sb[:, h, qs_ : qs_ + nq],
                                start=True,
                                stop=True,
                            )
                            if causal:
                                if qs_ == q0:  # diagonal block
                                    nc.vector.tensor_add(
                                        sc[:, 0:128], sc[:, 0:128], mt_sb[:]
                                    )
                            else:
                                nc.vector.tensor_add(
                                    sc[:, :nq], sc[:, :nq], mt_sb[:, ki, qs_ : qs_ + nq]
                                )
                            po = offs[ki] + qs_ - q0
                            nc.scalar.activation(
                                pT[:, po : po + nq],
                                sc[:, :nq],
                                mybir.ActivationFunctionType.Exp,
                                scale=SQ,
                            )
                            # denominators accumulate in PSUM across ki; the
                            # causal q-ranges nest, so ki==0 (full range)
                            # starts the group for every column.
                            nc.tensor.matmul(
                                sums[0:1, qs_ : qs_ + nq],
                                ones_sb[:],
                                pT[:, po : po + nq],
                                start=(ki == 0),
                                stop=(ki == nkb - 1),
                                skip_group_check=True,
                            )
                    # denominator chain, split into <=512 column pieces so each
                    # serial stage is short and pieces pipeline across engines
                    nhalf = (s + 511) // 512
                    width = s // nhalf
                    rbrs = []
                    for hs in range(nhalf):
                        ssb = small.tile([1, width], F32, tag="ssb", bufs=4, name=f"ssb_{b}_{h}_{hs}")
                        nc.scalar.copy(ssb[0:1, :], sums[0:1, hs * width : (hs + 1) * width])
                        rb = small.tile([128, width], F32, tag="rb", bufs=4, name=f"rb_{b}_{h}_{hs}")
                        nc.gpsimd.partition_broadcast(rb[:], ssb[0:1, :])
                        rbr = small.tile([128, width], F32, tag="rbr", bufs=4, name=f"rbr_{b}_{h}_{hs}")
                        nc.vector.reciprocal_approx_fast(rbr[:], rb[:])
                        rbrs.append(rbr)
                    return pT, rbrs, width

                def attn_pv(h):
                    pT, rbrs, width = stage1.pop(h)
                    for qi in range(nkb):
                        o_ps = pss.tile([128, HD], F32, tag="sm", name=f"ops_{b}_{h}_{qi}")
                        kis = [k for k in range(nkb) if (not causal) or k <= qi]
                        for j, ki in enumerate(kis):
                            nc.tensor.matmul(
                                o_ps[:],
                                v_sb[:, ki, :],
                                pT[:, offs[ki] + qi * 128 - q0s[ki] : offs[ki] + qi * 128 - q0s[ki] + 128],
                                start=(j == 0),
                                stop=(j == len(kis) - 1),
                            )
                        nc.vector.tensor_mul(
                            attnT_sb[:, h, qi * 128 : (qi + 1) * 128],
                            o_ps[:],
                            rbrs[qi * 128 // width][:, qi * 128 % width : qi * 128 % width + 128],
                        )

                for h in range(NH):
                    stage1[h] = attn_scores(h)
                    if h > 0:
                        attn_pv(h - 1)
                attn_pv(NH - 1)

                # ---- phase 4: output projection (partial over wo rows) --
                for nb in range(d // 512):
                    wo_nb = wopool.tile([128, NH, 512], BF16, tag="wo", name=f"wo_{b}_{nb}")
                    nc.sync.dma_start(
                        wo_nb[:],
                        wo.rearrange("(h p) n -> p h n", p=128)[
                            :, :, nb * 512 : (nb + 1) * 512
                        ],
                    )
                    for tp in range(nkb // 2):
                        ot = oev.tile([128, 2, 512], F32, tag="ot", name=f"ot_{b}_{nb}_{tp}")
                        for half in range(2):
                            tbk = tp * 2 + half
                            o2 = psm.tile([128, 512], F32, tag="mm", name=f"o2_{b}_{nb}_{tbk}")
                            for h in range(NH):
                                nc.tensor.matmul(
                                    o2[:],
                                    attnT_sb[:, h, tbk * 128 : (tbk + 1) * 128],
                                    wo_nb[:, h, :],
                                    start=(h == 0),
                                    stop=(h == NH - 1),
                                )
                            if half == 0:
                                nc.scalar.copy(ot[:, half, :], o2[:])
                            else:
                                nc.vector.tensor_copy(ot[:, half, :], o2[:])
                        nc.sync.dma_start(
                            out[
                                b * s + tp * 256 : b * s + (tp + 1) * 256,
                                nb * 512 : (nb + 1) * 512,
                            ].rearrange("(rh p) n -> p rh n", p=128),
                            ot[:],
                        )
    nc.compile()
    return nc


# ---------------------------------------------------------------------------
# host side
# ---------------------------------------------------------------------------

_PERM = np.concatenate([np.arange(0, HD, 2), np.arange(1, HD, 2)])
_CACHE = {}


def _get_program(causal):
    if causal not in _CACHE:
        _CACHE[causal] = build_program(causal)
    return _CACHE[causal]


def _is_causal(mask):
    iu = np.triu_indices(S, 1)
    il = np.tril_indices(S)
    return bool(np.all(mask[il] == 0.0) and np.all(mask[iu] < -1e8))


def make_in_maps(x, cos, sin, mask, wq, wk, wv, wo, causal):
    x = np.asarray(x, dtype=np.float32)
    cos = np.asarray(cos, dtype=np.float32)
    sin = np.asarray(sin, dtype=np.float32)
    mask = np.asarray(mask, dtype=np.float32)
    wq = np.asarray(wq, dtype=np.float32)
    wk = np.asarray(wk, dtype=np.float32)
    wv = np.asarray(wv, dtype=np.float32)
    wo = np.asarray(wo, dtype=np.float32)

    xT = np.ascontiguousarray(x.reshape(B * S, D).T).astype(NPBF16)
    c2 = np.ascontiguousarray(np.concatenate([cos.T, cos.T], 0)).astype(np.float32)
    s2 = np.ascontiguousarray(np.concatenate([-sin.T, sin.T], 0)).astype(np.float32)
    swm = np.eye(128, dtype=np.float32).astype(NPBF16)  # transpose identity
    if causal:
        mt = np.ascontiguousarray(mask[:128, :128].T * math.sqrt(HD)).astype(np.float32)
    else:
        mt = np.ascontiguousarray(mask.T * math.sqrt(HD)).astype(NPBF16)

    in_maps = []
    for c in range(NCORES):
        wq_c = wq[:, c * NH * HD : (c + 1) * NH * HD].reshape(D, NH, HD)[:, :, _PERM]
        wq_c = np.ascontiguousarray(wq_c.reshape(D, NH * HD)).astype(NPBF16)
        wk_c = np.ascontiguousarray(wk[:, c * HD : (c + 1) * HD][:, _PERM]).astype(NPBF16)
        wv_c = np.ascontiguousarray(wv[:, c * HD : (c + 1) * HD]).astype(NPBF16)
        wo_c = np.ascontiguousarray(wo[c * NH * HD : (c + 1) * NH * HD, :]).astype(NPBF16)
        in_maps.append(
            {
                "xT": xT,
                "wq": wq_c,
                "wk": wk_c,
                "wv": wv_c,
                "wo": wo_c,
                "sw": swm,
                "c2": c2,
                "s2": s2,
                "mt": mt,
            }
        )
    return in_maps


def run(in_maps, causal, **kwargs):
    nc = _get_program(causal)
    return run_bass_kernel_spmd(nc, in_maps, core_ids=list(range(NCORES)), **kwargs)


def kernel(x, start_pos, cos, sin, mask, wq, wk, wv, wo):
    mask = np.asarray(mask, dtype=np.float32)
    causal = _is_causal(mask)
    in_maps = make_in_maps(x, cos, sin, mask, wq, wk, wv, wo, causal)
    res = run(in_maps, causal)
    acc = np.zeros((B * S, D), dtype=np.float32)
    for c in range(NCORES):
        acc += np.asarray(res.results[c]["out"], dtype=np.float32)
    return acc.reshape(B, S, D)


# revision 20
# speedup vs baseline: 1.4283x; 1.1097x over previous
"""GQA attention prefill (B=2, S=1024, D=4096, H=32, KVH=8, HD=128) on 8 TRN2
NeuronCores.

Sharding: tensor-parallel over heads. Core c owns KV head c and query heads
4c..4c+3 (GQA groups align with cores), i.e. column-shards of wq/wk/wv and the
matching row-shard of wo. Each core computes its partial `attn_c @ wo_c`
([B*S, D]); the host sums the 8 partials.

Device layouts (per core):
  xT   [D, B*S]   bf16  host-transposed activations (contraction dim on partitions)
  wq   [D, 512]   bf16  rope-permuted (even dims then odd dims within each head)
  wk   [D, 128]   bf16  rope-permuted
  wv   [D, 128]   bf16
  wo   [512, D]   bf16
  c2   [128, S]   f32   rope cos table, duplicated across the two 64-row halves
  s2   [128, S]   f32   rope sin table, [-sin; +sin]
  mt   [128,128]  f32   causal diagonal-block additive mask * sqrt(HD)   (causal)
  mt   [S, S]     bf16  full transposed additive mask * sqrt(HD)         (general)

Q/K are produced transposed ([d, tok]) straight out of the projection matmul;
scores are computed transposed ([k, q]) so softmax denominators come from a
ones-vector matmul and P^T feeds the PV matmul with no transposes anywhere.
Rope's even/odd pairing is turned into a contiguous half-swap by permuting the
weight columns; the swap itself is two SBUF->SBUF partition-block DMAs.
Softmax skips the max-subtraction (scores are O(10); exp accumulates in fp32).
"""

import math
from contextlib import ExitStack

import numpy as np
import ml_dtypes

import concourse.bass as bass
import concourse.mybir as mybir
import concourse.tile as tile
from concourse import bacc
from concourse.bass_utils import run_bass_kernel_spmd

BF16 = mybir.dt.bfloat16
F32 = mybir.dt.float32
NPBF16 = ml_dtypes.bfloat16

B, S, D, H, KVH, HD = 2, 1024, 4096, 32, 8, 128
NCORES = 8
NH = H // NCORES          # 4 query heads per core
DC = D // 128             # 32 contraction chunks
TB = 256                  # token chunk for the QKV projection
SQ = 1.0 / math.sqrt(HD)


def _chunks(q0, qend, step=512):
    qs = q0
    while qs < qend:
        nq = min(step, qend - qs)
        yield qs, nq
        qs += nq


def build_program(causal, s=S, d=D, tb=TB):
    """Build the per-core SPMD program. s/d/tb are overridable for sim tests."""
    dc = d // 128
    nkb = s // 128            # number of 128-wide key/query blocks per batch
    ntc = s // tb             # token chunks per batch
    qcols = NH * HD

    # pT packing offsets: causal keeps only k-block ki's valid q range [128ki, s)
    if causal:
        q0s = [ki * 128 for ki in range(nkb)]
    else:
        q0s = [0] * nkb
    offs, acc = [], 0
    for ki in range(nkb):
        offs.append(acc)
        acc += s - q0s[ki]
    pt_len = acc

    nc = bacc.Bacc(
        "TRN2",
        target_bir_lowering=False,
        debug=False,
        enable_asserts=False,
        num_devices=1,
    )
    xT = nc.dram_tensor("xT", [d, B * s], BF16, kind="ExternalInput").ap()
    wq = nc.dram_tensor("wq", [d, qcols], BF16, kind="ExternalInput").ap()
    wk = nc.dram_tensor("wk", [d, HD], BF16, kind="ExternalInput").ap()
    wv = nc.dram_tensor("wv", [d, HD], BF16, kind="ExternalInput").ap()
    wo = nc.dram_tensor("wo", [qcols, d], BF16, kind="ExternalInput").ap()
    sw = nc.dram_tensor("sw", [128, 128], BF16, kind="ExternalInput").ap()
    c2 = nc.dram_tensor("c2", [128, s], F32, kind="ExternalInput").ap()
    s2 = nc.dram_tensor("s2", [128, s], F32, kind="ExternalInput").ap()
    if causal:
        mt = nc.dram_tensor("mt", [128, 128], F32, kind="ExternalInput").ap()
    else:
        mt = nc.dram_tensor("mt", [s, s], BF16, kind="ExternalInput").ap()
    out = nc.dram_tensor("out", [B * s, d], BF16, kind="ExternalOutput").ap()

    with tile.TileContext(nc) as tc:
        with ExitStack() as ctx:
            const = ctx.enter_context(tc.tile_pool(name="const", bufs=1))
            xpool = ctx.enter_context(tc.tile_pool(name="xpool", bufs=2))
            wopool = ctx.enter_context(tc.tile_pool(name="wopool", bufs=2))
            qkv = ctx.enter_context(tc.tile_pool(name="qkv", bufs=2))
            ptp = ctx.enter_context(tc.tile_pool(name="ptp", bufs=2))
            rp = ctx.enter_context(tc.tile_pool(name="rp", bufs=3))
            small = ctx.enter_context(tc.tile_pool(name="small", bufs=2))
            oev = ctx.enter_context(tc.tile_pool(name="oev", bufs=2))
            psm = ctx.enter_context(tc.tile_pool(name="psm", bufs=3, space="PSUM"))
            pss = ctx.enter_context(tc.tile_pool(name="pss", bufs=3, space="PSUM"))

            # resident constants / weights
            c2_sb = const.tile([128, s], F32)
            nc.sync.dma_start(c2_sb[:], c2[:])
            s2_sb = const.tile([128, s], F32)
            nc.sync.dma_start(s2_sb[:], s2[:])
            if causal:
                mt_sb = const.tile([128, 128], F32)
                nc.sync.dma_start(mt_sb[:], mt[:])
            else:
                mt_sb = const.tile([128, nkb, s], BF16)
                nc.sync.dma_start(mt_sb[:], mt.rearrange("(kb p) q -> p kb q", p=128))
            ones_sb = const.tile([128, 1], BF16)
            nc.vector.memset(ones_sb[:], 1.0)
            id_sb = const.tile([128, 128], BF16)
            nc.gpsimd.dma_start(id_sb[:], sw[:])
            wq_sb = const.tile([128, dc, qcols], BF16)
            nc.sync.dma_start(wq_sb[:], wq.rearrange("(c p) m -> p c m", p=128))
            wk_sb = const.tile([128, dc, HD], BF16)
            nc.sync.dma_start(wk_sb[:], wk.rearrange("(c p) m -> p c m", p=128))
            wv_sb = const.tile([128, dc, HD], BF16)
            nc.sync.dma_start(wv_sb[:], wv.rearrange("(c p) m -> p c m", p=128))

            def rope(ps, tok0, w, out_slice):
                """ps: [128, w] psum with raw projected Q/K block (d-permuted).
                out = raw*c2 + halfswap(raw)*s2, written as bf16 to out_slice.
                Only the ACT eviction touches PSUM; the swap is two SBUF
                partition-block DMAs and the muls run from SBUF on gpsimd/DVE."""
                raw = rp.tile([128, w], BF16, tag="raw", name=f"raw_{tok0}")
                nc.scalar.copy(raw[:], ps[:, :w])
                swt = rp.tile([128, w], BF16, tag="swt", name=f"swt_{tok0}")
                nc.sync.dma_start(swt[0:64, :], raw[64:128, :])
                nc.sync.dma_start(swt[64:128, :], raw[0:64, :])
                t1 = rp.tile([128, w], F32, tag="t1", name=f"t1_{tok0}")
                nc.vector.tensor_mul(t1[:], swt[:], s2_sb[:, tok0 : tok0 + w])
                t2 = rp.tile([128, w], F32, tag="t2", name=f"t2_{tok0}")
                nc.vector.tensor_mul(t2[:], raw[:], c2_sb[:, tok0 : tok0 + w])
                nc.gpsimd.tensor_add(out_slice, t2[:], t1[:])

            for b in range(B):
                # ---- persistent per-batch activations -------------------
                qT_sb = qkv.tile([128, NH, s], BF16, tag="qT", name=f"qT_{b}")
                kT_sb = qkv.tile([128, s], BF16, tag="kT", name=f"kT_{b}")
                vT_sb = qkv.tile([128, s], BF16, tag="vT", name=f"vT_{b}")
                v_sb = qkv.tile([128, nkb, HD], BF16, tag="v", name=f"v_{b}")
                attnT_sb = qkv.tile([128, NH, s], BF16, tag="attnT", name=f"attnT_{b}")

                # ---- phase 1+2: stream xT, project Q/K/V ----------------
                # evictions/rope are emitted one projection late, so each
                # engine's FIFO only sees work whose PSUM inputs are (nearly)
                # ready — avoids head-of-line blocking behind matmul chains.
                pending = []

                def flush(keep):
                    while len(pending) > keep:
                        kind, ps, tok0_ = pending.pop(0)
                        if kind == "k":
                            rope(ps, tok0_, tb, kT_sb[:, tok0_ : tok0_ + tb])
                        elif kind.startswith("q"):
                            h = int(kind[1:])
                            rope(ps, tok0_, tb, qT_sb[:, h, tok0_ : tok0_ + tb])
                        else:  # vt
                            nc.scalar.copy(vT_sb[:, tok0_ : tok0_ + tb], ps[:])
                            for m2 in range(tb // 128):
                                kb = tok0_ // 128 + m2
                                vtp = pss.tile(
                                    [128, HD], BF16, tag="sm", name=f"vtp_{b}_{kb}"
                                )
                                nc.tensor.transpose(
                                    vtp[:], vT_sb[:, kb * 128 : (kb + 1) * 128], id_sb[:]
                                )
                                nc.scalar.copy(v_sb[:, kb, :], vtp[:])

                for t4 in range(ntc):
                    tok0 = t4 * tb
                    xc = xpool.tile([128, dc, tb], BF16, tag="xc", name=f"xc_{b}_{t4}")
                    nc.sync.dma_start(
                        xc[:],
                        xT[:, b * s + tok0 : b * s + tok0 + tb].rearrange(
                            "(c p) t -> p c t", p=128
                        ),
                    )
                    # K projection -> kT (transposed layout, rope applied)
                    k_ps = psm.tile([128, tb], F32, tag="mm", name=f"kps_{b}_{t4}")
                    for c in range(dc):
                        nc.tensor.matmul(
                            k_ps[:],
                            wk_sb[:, c, :],
                            xc[:, c, :],
                            start=(c == 0),
                            stop=(c == dc - 1),
                        )
                    pending.append(("k", k_ps, tok0))
                    flush(1)
                    # V projection, transposed like K (wide-N matmuls), then
                    # PE-transposed back to the natural [tok, d] layout
                    vt_ps = psm.tile([128, tb], F32, tag="mm", name=f"vtps_{b}_{t4}")
                    for c in range(dc):
                        nc.tensor.matmul(
                            vt_ps[:],
                            wv_sb[:, c, :],
                            xc[:, c, :],
                            start=(c == 0),
                            stop=(c == dc - 1),
                        )
                    pending.append(("vt", vt_ps, tok0))
                    flush(1)
                    # Q projection -> qT (transposed layout, rope applied)
                    for h in range(NH):
                        q_ps = psm.tile([128, tb], F32, tag="mm", name=f"qps_{b}_{t4}_{h}")
                        for c in range(dc):
                            nc.tensor.matmul(
                                q_ps[:],
                                wq_sb[:, c, h * 128 : (h + 1) * 128],
                                xc[:, c, :],
                                start=(c == 0),
                                stop=(c == dc - 1),
                            )
                        pending.append((f"q{h}", q_ps, tok0))
                        flush(1)
                flush(0)

                # ---- phase 3: attention per query head ------------------
                # software-pipelined: PV of head h-1 is emitted after the
                # scores/sums of head h, so the softmax-denominator chain of
                # head h-1 hides under head h's PE work.
                stage1 = {}

                def attn_scores(h):
                    pT = ptp.tile([128, pt_len], BF16, tag="pt", name=f"pt_{b}_{h}")
                    sums = pss.tile([1, s], F32, tag="sums", bufs=1, name=f"sums_{b}_{h}")
                    for ki in range(nkb):
                        q0 = q0s[ki]
                        for qs_, nq in _chunks(q0, s):
                            sc = psm.tile([128, 512], F32, tag="mm", name=f"sc_{b}_{h}_{ki}_{qs_}")
                            nc.tensor.matmul(
                                sc[:, :nq],
                                kT_sb[:, ki * 128 : (ki + 1) * 128],
                                qT_sb[:, h, qs_ : qs_ + nq],
                                start=True,
                                stop=True,
                            )
                            if causal:
                                if qs_ == q0:  # diagonal block
                                    nc.vector.tensor_add(
                                        sc[:, 0:128], sc[:, 0:128], mt_sb[:]
                                    )
                            else:
                                nc.vector.tensor_add(
                                    sc[:, :nq], sc[:, :nq], mt_sb[:, ki, qs_ : qs_ + nq]
                                )
                            po = offs[ki] + qs_ - q0
                            nc.scalar.activation(
                                pT[:, po : po + nq],
                                sc[:, :nq],
                                mybir.ActivationFunctionType.Exp,
                                scale=SQ,
                            )
                            # denominators accumulate in PSUM across ki; the
                            # causal q-ranges nest, so ki==0 (full range)
                            # starts the group for every column.
                            nc.tensor.matmul(
                                sums[0:1, qs_ : qs_ + nq],
                                ones_sb[:],
                                pT[:, po : po + nq],
                                start=(ki == 0),
                                stop=(ki == nkb - 1),
                                skip_group_check=True,
                            )
                    # denominator chain, split into <=512 column pieces so each
                    # serial stage is short and pieces pipeline across engines
                    nhalf = (s + 511) // 512
                    width = s // nhalf
                    rbrs = []
                    for hs in range(nhalf):
                        ssb = small.tile([1, width], F32, tag="ssb", bufs=4, name=f"ssb_{b}_{h}_{hs}")
                        nc.scalar.copy(ssb[0:1, :], sums[0:1, hs * width : (hs + 1) * width])
                        rb = small.tile([128, width], F32, tag="rb", bufs=4, name=f"rb_{b}_{h}_{hs}")
                        nc.gpsimd.partition_broadcast(rb[:], ssb[0:1, :])
                        rbr = small.tile([128, width], F32, tag="rbr", bufs=4, name=f"rbr_{b}_{h}_{hs}")
                        nc.vector.reciprocal_approx_fast(rbr[:], rb[:])
                        rbrs.append(rbr)
                    return pT, rbrs, width

                def attn_pv(h):
                    pT, rbrs, width = stage1.pop(h)
                    for qi in range(nkb):
                        o_ps = pss.tile([128, HD], F32, tag="sm", name=f"ops_{b}_{h}_{qi}")
                        kis = [k for k in range(nkb) if (not causal) or k <= qi]
                        for j, ki in enumerate(kis):
                            nc.tensor.matmul(
                                o_ps[:],
                                v_sb[:, ki, :],
                                pT[:, offs[ki] + qi * 128 - q0s[ki] : offs[ki] + qi * 128 - q0s[ki] + 128],
                                start=(j == 0),
                                stop=(j == len(kis) - 1),
                            )
                        nc.vector.tensor_mul(
                            attnT_sb[:, h, qi * 128 : (qi + 1) * 128],
                            o_ps[:],
                            rbrs[qi * 128 // width][:, qi * 128 % width : qi * 128 % width + 128],
                        )

                for h in range(NH):
                    stage1[h] = attn_scores(h)
                    if h > 0:
                        attn_pv(h - 1)
                attn_pv(NH - 1)

                # ---- phase 4: output projection (partial over wo rows) --
                for nb in range(d // 512):
                    wo_nb = wopool.tile([128, NH, 512], BF16, tag="wo", name=f"wo_{b}_{nb}")
                    nc.sync.dma_start(
                        wo_nb[:],
                        wo.rearrange("(h p) n -> p h n", p=128)[
                            :, :, nb * 512 : (nb + 1) * 512
                        ],
                    )
                    for tp in range(nkb // 2):
                        ot = oev.tile([128, 2, 512], BF16, tag="ot", bufs=3, name=f"ot_{b}_{nb}_{tp}")
                        for half in range(2):
                            tbk = tp * 2 + half
                            o2 = psm.tile([128, 512], F32, tag="mm", name=f"o2_{b}_{nb}_{tbk}")
                            for h in range(NH):
                                nc.tensor.matmul(
                                    o2[:],
                                    attnT_sb[:, h, tbk * 128 : (tbk + 1) * 128],
                                    wo_nb[:, h, :],
                                    start=(h == 0),
                                    stop=(h == NH - 1),
                                )
                            if half == 0:
                                nc.scalar.copy(ot[:, half, :], o2[:])
                            else:
                                nc.vector.tensor_copy(ot[:, half, :], o2[:])
                        nc.sync.dma_start(
                            out[
                                b * s + tp * 256 : b * s + (tp + 1) * 256,
                                nb * 512 : (nb + 1) * 512,
                            ].rearrange("(rh p) n -> p rh n", p=128),
                            ot[:],
                        )
    nc.compile()
    return nc


# ---------------------------------------------------------------------------
# host side
# ---------------------------------------------------------------------------

_PERM = np.concatenate([np.arange(0, HD, 2), np.arange(1, HD, 2)])
_CACHE = {}


def _get_program(causal):
    if causal not in _CACHE:
        _CACHE[causal] = build_program(causal)
    return _CACHE[causal]


def _is_causal(mask):
    iu = np.triu_indices(S, 1)
    il = np.tril_indices(S)
    return bool(np.all(mask[il] == 0.0) and np.all(mask[iu] < -1e8))


def make_in_maps(x, cos, sin, mask, wq, wk, wv, wo, causal):
    x = np.asarray(x, dtype=np.float32)
    cos = np.asarray(cos, dtype=np.float32)
    sin = np.asarray(sin, dtype=np.float32)
    mask = np.asarray(mask, dtype=np.float32)
    wq = np.asarray(wq, dtype=np.float32)
    wk = np.asarray(wk, dtype=np.float32)
    wv = np.asarray(wv, dtype=np.float32)
    wo = np.asarray(wo, dtype=np.float32)

    xT = np.ascontiguousarray(x.reshape(B * S, D).T).astype(NPBF16)
    c2 = np.ascontiguousarray(np.concatenate([cos.T, cos.T], 0)).astype(np.float32)
    s2 = np.ascontiguousarray(np.concatenate([-sin.T, sin.T], 0)).astype(np.float32)
    swm = np.eye(128, dtype=np.float32).astype(NPBF16)  # transpose identity
    if causal:
        mt = np.ascontiguousarray(mask[:128, :128].T * math.sqrt(HD)).astype(np.float32)
    else:
        mt = np.ascontiguousarray(mask.T * math.sqrt(HD)).astype(NPBF16)

    in_maps = []
    for c in range(NCORES):
        wq_c = wq[:, c * NH * HD : (c + 1) * NH * HD].reshape(D, NH, HD)[:, :, _PERM]
        wq_c = np.ascontiguousarray(wq_c.reshape(D, NH * HD)).astype(NPBF16)
        wk_c = np.ascontiguousarray(wk[:, c * HD : (c + 1) * HD][:, _PERM]).astype(NPBF16)
        wv_c = np.ascontiguousarray(wv[:, c * HD : (c + 1) * HD]).astype(NPBF16)
        wo_c = np.ascontiguousarray(wo[c * NH * HD : (c + 1) * NH * HD, :]).astype(NPBF16)
        in_maps.append(
            {
                "xT": xT,
                "wq": wq_c,
                "wk": wk_c,
                "wv": wv_c,
                "wo": wo_c,
                "sw": swm,
                "c2": c2,
                "s2": s2,
                "mt": mt,
            }
        )
    return in_maps


def run(in_maps, causal, **kwargs):
    nc = _get_program(causal)
    return run_bass_kernel_spmd(nc, in_maps, core_ids=list(range(NCORES)), **kwargs)


def kernel(x, start_pos, cos, sin, mask, wq, wk, wv, wo):
    mask = np.asarray(mask, dtype=np.float32)
    causal = _is_causal(mask)
    in_maps = make_in_maps(x, cos, sin, mask, wq, wk, wv, wo, causal)
    res = run(in_maps, causal)
    acc = np.zeros((B * S, D), dtype=np.float32)
    for c in range(NCORES):
        acc += np.asarray(res.results[c]["out"], dtype=np.float32)
    return acc.reshape(B, S, D)


# revision 26
# speedup vs baseline: 1.4385x; 1.0072x over previous
"""GQA attention prefill (B=2, S=1024, D=4096, H=32, KVH=8, HD=128) on 8 TRN2
NeuronCores.

Sharding: tensor-parallel over heads. Core c owns KV head c and query heads
4c..4c+3 (GQA groups align with cores), i.e. column-shards of wq/wk/wv and the
matching row-shard of wo. Each core computes its partial `attn_c @ wo_c`
([B*S, D]); the host sums the 8 partials.

Device layouts (per core):
  xT   [D, B*S]   bf16  host-transposed activations (contraction dim on partitions)
  wq   [D, 512]   bf16  rope-permuted (even dims then odd dims within each head)
  wk   [D, 128]   bf16  rope-permuted
  wv   [D, 128]   bf16
  wo   [512, D]   bf16
  c2   [128, S]   f32   rope cos table, duplicated across the two 64-row halves
  s2   [128, S]   f32   rope sin table, [-sin; +sin]
  mt   [128,128]  f32   causal diagonal-block additive mask * sqrt(HD)   (causal)
  mt   [S, S]     bf16  full transposed additive mask * sqrt(HD)         (general)

Q/K are produced transposed ([d, tok]) straight out of the projection matmul;
scores are computed transposed ([k, q]) so softmax denominators come from a
ones-vector matmul and P^T feeds the PV matmul with no transposes anywhere.
Rope's even/odd pairing is turned into a contiguous half-swap by permuting the
weight columns; the swap itself is two SBUF->SBUF partition-block DMAs.
Softmax skips the max-subtraction (scores are O(10); exp accumulates in fp32).
"""

import math
from contextlib import ExitStack

import numpy as np
import ml_dtypes

import concourse.bass as bass
import concourse.mybir as mybir
import concourse.tile as tile
from concourse import bacc
from concourse.bass_utils import run_bass_kernel_spmd

BF16 = mybir.dt.bfloat16
F32 = mybir.dt.float32
NPBF16 = ml_dtypes.bfloat16

B, S, D, H, KVH, HD = 2, 1024, 4096, 32, 8, 128
NCORES = 8
NH = H // NCORES          # 4 query heads per core
DC = D // 128             # 32 contraction chunks
TB = 256                  # token chunk for the QKV projection
SQ = 1.0 / math.sqrt(HD)


def _chunks(q0, qend, step=512):
    qs = q0
    while qs < qend:
        nq = min(step, qend - qs)
        yield qs, nq
        qs += nq


def build_program(causal, s=S, d=D, tb=TB):
    """Build the per-core SPMD program. s/d/tb are overridable for sim tests."""
    dc = d // 128
    nkb = s // 128            # number of 128-wide key/query blocks per batch
    ntc = s // tb             # token chunks per batch
    qcols = NH * HD

    # pT packing offsets: causal keeps only k-block ki's valid q range [128ki, s)
    if causal:
        q0s = [ki * 128 for ki in range(nkb)]
    else:
        q0s = [0] * nkb
    offs, acc = [], 0
    for ki in range(nkb):
        offs.append(acc)
        acc += s - q0s[ki]
    pt_len = acc

    nc = bacc.Bacc(
        "TRN2",
        target_bir_lowering=False,
        debug=False,
        enable_asserts=False,
        num_devices=1,
    )
    xT = nc.dram_tensor("xT", [d, B * s], BF16, kind="ExternalInput").ap()
    wq = nc.dram_tensor("wq", [d, qcols], BF16, kind="ExternalInput").ap()
    wk = nc.dram_tensor("wk", [d, HD], BF16, kind="ExternalInput").ap()
    wv = nc.dram_tensor("wv", [d, HD], BF16, kind="ExternalInput").ap()
    wo = nc.dram_tensor("wo", [qcols, d], BF16, kind="ExternalInput").ap()
    sw = nc.dram_tensor("sw", [128, 128], BF16, kind="ExternalInput").ap()
    c2 = nc.dram_tensor("c2", [128, s], F32, kind="ExternalInput").ap()
    s2 = nc.dram_tensor("s2", [128, s], F32, kind="ExternalInput").ap()
    if causal:
        mt = nc.dram_tensor("mt", [128, 128], F32, kind="ExternalInput").ap()
    else:
        mt = nc.dram_tensor("mt", [s, s], BF16, kind="ExternalInput").ap()
    out = nc.dram_tensor("out", [B * s, d], BF16, kind="ExternalOutput").ap()

    with tile.TileContext(nc) as tc:
        with ExitStack() as ctx:
            const = ctx.enter_context(tc.tile_pool(name="const", bufs=1))
            xpool = ctx.enter_context(tc.tile_pool(name="xpool", bufs=2))
            wopool = ctx.enter_context(tc.tile_pool(name="wopool", bufs=2))
            qkv = ctx.enter_context(tc.tile_pool(name="qkv", bufs=2))
            ptp = ctx.enter_context(tc.tile_pool(name="ptp", bufs=2))
            rp = ctx.enter_context(tc.tile_pool(name="rp", bufs=3))
            small = ctx.enter_context(tc.tile_pool(name="small", bufs=2))
            oev = ctx.enter_context(tc.tile_pool(name="oev", bufs=2))
            psm = ctx.enter_context(tc.tile_pool(name="psm", bufs=3, space="PSUM"))
            pss = ctx.enter_context(tc.tile_pool(name="pss", bufs=3, space="PSUM"))

            # resident constants / weights
            c2_sb = const.tile([128, s], F32)
            nc.sync.dma_start(c2_sb[:], c2[:])
            s2_sb = const.tile([128, s], F32)
            nc.sync.dma_start(s2_sb[:], s2[:])
            if causal:
                mt_sb = const.tile([128, 128], F32)
                nc.sync.dma_start(mt_sb[:], mt[:])
            else:
                mt_sb = const.tile([128, nkb, s], BF16)
                nc.sync.dma_start(mt_sb[:], mt.rearrange("(kb p) q -> p kb q", p=128))
            ones_sb = const.tile([128, 1], BF16)
            nc.vector.memset(ones_sb[:], 1.0)
            id_sb = const.tile([128, 128], BF16)
            nc.gpsimd.dma_start(id_sb[:], sw[:])
            wq_sb = const.tile([128, dc, qcols], BF16)
            nc.sync.dma_start(wq_sb[:], wq.rearrange("(c p) m -> p c m", p=128))
            wk_sb = const.tile([128, dc, HD], BF16)
            nc.sync.dma_start(wk_sb[:], wk.rearrange("(c p) m -> p c m", p=128))
            wv_sb = const.tile([128, dc, HD], BF16)
            nc.sync.dma_start(wv_sb[:], wv.rearrange("(c p) m -> p c m", p=128))

            def rope(ps, tok0, w, out_slice):
                """ps: [128, w] psum with raw projected Q/K block (d-permuted).
                out = raw*c2 + halfswap(raw)*s2, written as bf16 to out_slice.
                Only the ACT eviction touches PSUM; the swap is two SBUF
                partition-block DMAs and the muls run from SBUF on gpsimd/DVE."""
                raw = rp.tile([128, w], BF16, tag="raw", name=f"raw_{tok0}")
                nc.scalar.copy(raw[:], ps[:, :w])
                swt = rp.tile([128, w], BF16, tag="swt", name=f"swt_{tok0}")
                nc.sync.dma_start(swt[0:64, :], raw[64:128, :])
                nc.sync.dma_start(swt[64:128, :], raw[0:64, :])
                t1 = rp.tile([128, w], F32, tag="t1", name=f"t1_{tok0}")
                nc.vector.tensor_mul(t1[:], swt[:], s2_sb[:, tok0 : tok0 + w])
                t2 = rp.tile([128, w], F32, tag="t2", name=f"t2_{tok0}")
                nc.vector.tensor_mul(t2[:], raw[:], c2_sb[:, tok0 : tok0 + w])
                nc.gpsimd.tensor_add(out_slice, t2[:], t1[:])

            def phase2(b):
                """Stream xT, project Q/K/V for batch b. Returns the
                per-batch activation tiles."""
                qT_sb = qkv.tile([128, NH, s], BF16, tag="qT", name=f"qT_{b}")
                kT_sb = qkv.tile([128, s], BF16, tag="kT", name=f"kT_{b}")
                vT_sb = qkv.tile([128, s], BF16, tag="vT", name=f"vT_{b}")
                v_sb = qkv.tile([128, nkb, HD], BF16, tag="v", name=f"v_{b}")
                attnT_sb = qkv.tile([128, NH, s], BF16, tag="attnT", name=f"attnT_{b}")

                # evictions/rope are emitted one projection late, so each
                # engine's FIFO only sees work whose PSUM inputs are (nearly)
                # ready — avoids head-of-line blocking behind matmul chains.
                pending = []

                def flush(keep):
                    while len(pending) > keep:
                        kind, ps, tok0_ = pending.pop(0)
                        if kind == "k":
                            rope(ps, tok0_, tb, kT_sb[:, tok0_ : tok0_ + tb])
                        elif kind.startswith("q"):
                            h = int(kind[1:])
                            rope(ps, tok0_, tb, qT_sb[:, h, tok0_ : tok0_ + tb])
                        else:  # vt
                            nc.vector.tensor_copy(vT_sb[:, tok0_ : tok0_ + tb], ps[:])
                            for m2 in range(tb // 128):
                                kb = tok0_ // 128 + m2
                                vtp = pss.tile(
                                    [128, HD], BF16, tag="sm", name=f"vtp_{b}_{kb}"
                                )
                                nc.tensor.transpose(
                                    vtp[:], vT_sb[:, kb * 128 : (kb + 1) * 128], id_sb[:]
                                )
                                nc.scalar.copy(v_sb[:, kb, :], vtp[:])

                for t4 in range(ntc):
                    tok0 = t4 * tb
                    xc = xpool.tile([128, dc, tb], BF16, tag="xc", name=f"xc_{b}_{t4}")
                    nc.sync.dma_start(
                        xc[:],
                        xT[:, b * s + tok0 : b * s + tok0 + tb].rearrange(
                            "(c p) t -> p c t", p=128
                        ),
                    )
                    # K projection -> kT (transposed layout, rope applied)
                    k_ps = psm.tile([128, tb], F32, tag="mm", name=f"kps_{b}_{t4}")
                    for c in range(dc):
                        nc.tensor.matmul(
                            k_ps[:],
                            wk_sb[:, c, :],
                            xc[:, c, :],
                            start=(c == 0),
                            stop=(c == dc - 1),
                        )
                    pending.append(("k", k_ps, tok0))
                    flush(1)
                    # V projection, transposed like K (wide-N matmuls), then
                    # PE-transposed back to the natural [tok, d] layout
                    vt_ps = psm.tile([128, tb], F32, tag="mm", name=f"vtps_{b}_{t4}")
                    for c in range(dc):
                        nc.tensor.matmul(
                            vt_ps[:],
                            wv_sb[:, c, :],
                            xc[:, c, :],
                            start=(c == 0),
                            stop=(c == dc - 1),
                        )
                    pending.append(("vt", vt_ps, tok0))
                    flush(1)
                    # Q projection -> qT (transposed layout, rope applied)
                    for h in range(NH):
                        q_ps = psm.tile([128, tb], F32, tag="mm", name=f"qps_{b}_{t4}_{h}")
                        for c in range(dc):
                            nc.tensor.matmul(
                                q_ps[:],
                                wq_sb[:, c, h * 128 : (h + 1) * 128],
                                xc[:, c, :],
                                start=(c == 0),
                                stop=(c == dc - 1),
                            )
                        pending.append((f"q{h}", q_ps, tok0))
                        flush(1)
                flush(0)
                return dict(qT=qT_sb, kT=kT_sb, vT=vT_sb, v=v_sb, attnT=attnT_sb)

            def attn(b, T):
                """Attention for batch b, software-pipelined: PV of head h-1
                is emitted after the scores/sums of head h, so the softmax-
                denominator chain of head h-1 hides under head h's PE work."""
                qT_sb, kT_sb, v_sb, attnT_sb = T["qT"], T["kT"], T["v"], T["attnT"]
                stage1 = {}

                def attn_scores(h):
                    pT = ptp.tile([128, pt_len], BF16, tag="pt", name=f"pt_{b}_{h}")
                    sums = pss.tile([1, s], F32, tag="sums", bufs=1, name=f"sums_{b}_{h}")
                    for ki in range(nkb):
                        q0 = q0s[ki]
                        for qs_, nq in _chunks(q0, s):
                            sc = psm.tile([128, 512], F32, tag="mm", name=f"sc_{b}_{h}_{ki}_{qs_}")
                            nc.tensor.matmul(
                                sc[:, :nq],
                                kT_sb[:, ki * 128 : (ki + 1) * 128],
                                qT_sb[:, h, qs_ : qs_ + nq],
                                start=True,
                                stop=True,
                            )
                            if causal:
                                if qs_ == q0:  # diagonal block
                                    nc.vector.tensor_add(
                                        sc[:, 0:128], sc[:, 0:128], mt_sb[:]
                                    )
                            else:
                                nc.vector.tensor_add(
                                    sc[:, :nq], sc[:, :nq], mt_sb[:, ki, qs_ : qs_ + nq]
                                )
                            po = offs[ki] + qs_ - q0
                            nc.scalar.activation(
                                pT[:, po : po + nq],
                                sc[:, :nq],
                                mybir.ActivationFunctionType.Exp,
                                scale=SQ,
                            )
                            # denominators accumulate in PSUM across ki; the
                            # causal q-ranges nest, so ki==0 (full range)
                            # starts the group for every column.
                            nc.tensor.matmul(
                                sums[0:1, qs_ : qs_ + nq],
                                ones_sb[:],
                                pT[:, po : po + nq],
                                start=(ki == 0),
                                stop=(ki == nkb - 1),
                                skip_group_check=True,
                            )
                    # denominator chain, split into <=512 column pieces so each
                    # serial stage is short and pieces pipeline across engines
                    nhalf = (s + 511) // 512
                    width = s // nhalf
                    rbrs = []
                    for hs in range(nhalf):
                        ssb = small.tile([1, width], F32, tag="ssb", bufs=4, name=f"ssb_{b}_{h}_{hs}")
                        nc.scalar.copy(ssb[0:1, :], sums[0:1, hs * width : (hs + 1) * width])
                        rb = small.tile([128, width], F32, tag="rb", bufs=4, name=f"rb_{b}_{h}_{hs}")
                        nc.gpsimd.partition_broadcast(rb[:], ssb[0:1, :])
                        rbr = small.tile([128, width], F32, tag="rbr", bufs=4, name=f"rbr_{b}_{h}_{hs}")
                        nc.vector.reciprocal_approx_fast(rbr[:], rb[:])
                        rbrs.append(rbr)
                    return pT, rbrs, width

                def attn_pv(h):
                    pT, rbrs, width = stage1.pop(h)
                    for qi in range(nkb):
                        o_ps = pss.tile([128, HD], F32, tag="sm", name=f"ops_{b}_{h}_{qi}")
                        kis = [k for k in range(nkb) if (not causal) or k <= qi]
                        for j, ki in enumerate(kis):
                            nc.tensor.matmul(
                                o_ps[:],
                                v_sb[:, ki, :],
                                pT[:, offs[ki] + qi * 128 - q0s[ki] : offs[ki] + qi * 128 - q0s[ki] + 128],
                                start=(j == 0),
                                stop=(j == len(kis) - 1),
                            )
                        nc.vector.tensor_mul(
                            attnT_sb[:, h, qi * 128 : (qi + 1) * 128],
                            o_ps[:],
                            rbrs[qi * 128 // width][:, qi * 128 % width : qi * 128 % width + 128],
                        )

                for h in range(NH):
                    stage1[h] = attn_scores(h)
                    if h > 0:
                        attn_pv(h - 1)
                attn_pv(NH - 1)

            def oproj(b, T):
                """Output projection (partial over this core's wo rows)."""
                attnT_sb = T["attnT"]
                for nb in range(d // 512):
                    wo_nb = wopool.tile([128, NH, 512], BF16, tag="wo", name=f"wo_{b}_{nb}")
                    nc.sync.dma_start(
                        wo_nb[:],
                        wo.rearrange("(h p) n -> p h n", p=128)[
                            :, :, nb * 512 : (nb + 1) * 512
                        ],
                    )
                    for tp in range(nkb // 2):
                        ot = oev.tile([128, 2, 512], BF16, tag="ot", bufs=3, name=f"ot_{b}_{nb}_{tp}")
                        for half in range(2):
                            tbk = tp * 2 + half
                            o2 = psm.tile([128, 512], F32, tag="mm", name=f"o2_{b}_{nb}_{tbk}")
                            for h in range(NH):
                                nc.tensor.matmul(
                                    o2[:],
                                    attnT_sb[:, h, tbk * 128 : (tbk + 1) * 128],
                                    wo_nb[:, h, :],
                                    start=(h == 0),
                                    stop=(h == NH - 1),
                                )
                            if half == 0:
                                nc.scalar.copy(ot[:, half, :], o2[:])
                            else:
                                nc.vector.tensor_copy(ot[:, half, :], o2[:])
                        nc.sync.dma_start(
                            out[
                                b * s + tp * 256 : b * s + (tp + 1) * 256,
                                nb * 512 : (nb + 1) * 512,
                            ].rearrange("(rh p) n -> p rh n", p=128),
                            ot[:],
                        )

            # phase order: batch b+1's projections are emitted before batch
            # b's output projection, so b's attention tail (softmax chains)
            # hides under b+1's dense matmul stream.
            T0 = phase2(0)
            attn(0, T0)
            if B > 1:
                T1 = phase2(1)
                oproj(0, T0)
                attn(1, T1)
                oproj(1, T1)
            else:
                oproj(0, T0)
    nc.compile()
    return nc


# ---------------------------------------------------------------------------
# host side
# ---------------------------------------------------------------------------

_PERM = np.concatenate([np.arange(0, HD, 2), np.arange(1, HD, 2)])
_CACHE = {}


def _get_program(causal):
    if causal not in _CACHE:
        _CACHE[causal] = build_program(causal)
    return _CACHE[causal]


def _is_causal(mask):
    iu = np.triu_indices(S, 1)
    il = np.tril_indices(S)
    return bool(np.all(mask[il] == 0.0) and np.all(mask[iu] < -1e8))


def make_in_maps(x, cos, sin, mask, wq, wk, wv, wo, causal):
    x = np.asarray(x, dtype=np.float32)
    cos = np.asarray(cos, dtype=np.float32)
    sin = np.asarray(sin, dtype=np.float32)
    mask = np.asarray(mask, dtype=np.float32)
    wq = np.asarray(wq, dtype=np.float32)
    wk = np.asarray(wk, dtype=np.float32)
    wv = np.asarray(wv, dtype=np.float32)
    wo = np.asarray(wo, dtype=np.float32)

    xT = np.ascontiguousarray(x.reshape(B * S, D).T).astype(NPBF16)
    c2 = np.ascontiguousarray(np.concatenate([cos.T, cos.T], 0)).astype(np.float32)
    s2 = np.ascontiguousarray(np.concatenate([-sin.T, sin.T], 0)).astype(np.float32)
    swm = np.eye(128, dtype=np.float32).astype(NPBF16)  # transpose identity
    if causal:
        mt = np.ascontiguousarray(mask[:128, :128].T * math.sqrt(HD)).astype(np.float32)
    else:
        mt = np.ascontiguousarray(mask.T * math.sqrt(HD)).astype(NPBF16)

    in_maps = []
    for c in range(NCORES):
        wq_c = wq[:, c * NH * HD : (c + 1) * NH * HD].reshape(D, NH, HD)[:, :, _PERM]
        wq_c = np.ascontiguousarray(wq_c.reshape(D, NH * HD)).astype(NPBF16)
        wk_c = np.ascontiguousarray(wk[:, c * HD : (c + 1) * HD][:, _PERM]).astype(NPBF16)
        wv_c = np.ascontiguousarray(wv[:, c * HD : (c + 1) * HD]).astype(NPBF16)
        wo_c = np.ascontiguousarray(wo[c * NH * HD : (c + 1) * NH * HD, :]).astype(NPBF16)
        in_maps.append(
            {
                "xT": xT,
                "wq": wq_c,
                "wk": wk_c,
                "wv": wv_c,
                "wo": wo_c,
                "sw": swm,
                "c2": c2,
                "s2": s2,
                "mt": mt,
            }
        )
    return in_maps


def run(in_maps, causal, **kwargs):
    nc = _get_program(causal)
    return run_bass_kernel_spmd(nc, in_maps, core_ids=list(range(NCORES)), **kwargs)


def kernel(x, start_pos, cos, sin, mask, wq, wk, wv, wo):
    mask = np.asarray(mask, dtype=np.float32)
    causal = _is_causal(mask)
    in_maps = make_in_maps(x, cos, sin, mask, wq, wk, wv, wo, causal)
    res = run(in_maps, causal)
    acc = np.zeros((B * S, D), dtype=np.float32)
    for c in range(NCORES):
        acc += np.asarray(res.results[c]["out"], dtype=np.float32)
    return acc.reshape(B, S, D)
